# revision 29
# baseline (speedup 1.0000x reference)
"""GroupedExperts (MoE bmm path) forward on 8 Trainium2 NeuronCores — v3.

Per expert e (one core each):
    h   = silu(x[e] @ w1[e]) * (x[e] @ w3[e]);  out = h @ w2[e]
E=8, T=4096, D=2048, H=1024, fp32 interface; bf16 on device.

v3 vs v2:
  - Chunked weight/x tiles with fine-grained DMA dependencies so the PE
    starts ~3us into the kernel instead of waiting ~53us for the full
    14MB preload: w1/w3 are host-packed hm-major ([p, (hm, k, h128)])
    and DMA'd as 8 x 512KB chunks each, interleaved so chunk hm arrives
    just before the PE's hm-th accumulation group needs it.
  - x blocks DMA'd as 4 x 512KB chunks per 512-token block (per-k-group
    dependency) instead of one 2MB tile.
  - w2 DMA'd as 8 per-hm chunks after w1/w3 (first needed ~55us in).

v4 vs v3:
  - PE warmup: a chain of 64 dummy 128x128 matmuls on a zeroed SBUF
    tile bridges the ~5.7us DMA wait after the ~6.8us Tile start
    barrier, so the HAM clock gate (activity-monitor throttle, 1.2 vs
    2.4 GHz) flips to full clock during the dummy chain and the real
    matmul stream starts warm (v3 paid ~60 cold matmuls, ~3.3us, plus
    ragged ramp gaps).
  - PSUM banks rebalanced 3(pa)+2(pb)+2(po)+1(warmup) = 8.
  - Last row block stores out per-512-col chunk, and the final 512-col
    accumulation group is split in two, so the cast+store chain after
    the very last matmul is as short as possible.

All DMAs stay on the single Sync HWDGE queue in exact PE-consumption
order: splitting across the Sync+Scalar queues was tried and is WORSE
(arrival order scrambles; w1/w3 chunks starve behind x/w2 and the PE
stalls ~10us mid-stream).

Measured (neuron-profile, max over 8 cores): ~683us vs ~663us pure
matmul floor (3072 MMs x 216ns); the rest is the Tile start barrier
(~6.8us), first-chunk DMA wait (~5.7us, bridged by warmup), drain/
barrier tail (~11us), and a ~10us periodic stall artifact (+163ns
every ~10.6us, present with wait-time 0; likely HBM refresh or
profiling overhead — not kernel-controllable).
"""

import numpy as np
import ml_dtypes

import concourse.mybir as mybir
import concourse.tile as tile
from concourse import bacc
from concourse.bass_utils import run_bass_kernel_spmd

E, T, D, H = 8, 4096, 2048, 1024
NCORES = 8
P = 128
TBLK = 512
NTBLK = T // TBLK          # 8 row blocks of 512 tokens
NDK = D // P               # 16 contraction chunks (d)
NHM = H // P               # 8 h blocks of 128
DBLK = 512
NDN = D // DBLK            # 4 output col blocks
NTSUB = TBLK // P          # 4 psum row blocks per row block
XCHUNK = 4                 # x chunks per row block (k groups of 4)
KPC = NDK // XCHUNK        # 4 k per x chunk
XCOLS = NTBLK * NDK * TBLK     # 65536
WCOLS = NDK * H                # 16384 (w1/w3); NHM * D = 16384 (w2)
WCHC = NDK * P                 # 2048 cols per w1/w3 hm chunk
W2CHC = D                      # 2048 cols per w2 hm chunk

BF16 = mybir.dt.bfloat16
F32 = mybir.dt.float32

_CACHE: dict = {}


def _build_module():
    if "nc" in _CACHE:
        return _CACHE["nc"]

    nc = bacc.Bacc(
        "TRN2",
        target_bir_lowering=False,
        debug=False,
        enable_asserts=False,
        num_devices=NCORES,
    )

    xp_d = nc.dram_tensor("xp", [P, XCOLS], BF16, kind="ExternalInput").ap()
    w1_d = nc.dram_tensor("w1", [P, WCOLS], BF16, kind="ExternalInput").ap()
    w3_d = nc.dram_tensor("w3", [P, WCOLS], BF16, kind="ExternalInput").ap()
    w2_d = nc.dram_tensor("w2", [P, WCOLS], BF16, kind="ExternalInput").ap()
    out_d = nc.dram_tensor("out", [T, D], BF16, kind="ExternalOutput").ap()

    with tile.TileContext(nc) as tc:
        with (
            tc.tile_pool(name="wpool", bufs=1) as wpool,
            tc.tile_pool(name="xpool", bufs=2 * XCHUNK) as xpool,
            tc.tile_pool(name="hpool", bufs=2) as hpool,
            tc.tile_pool(name="spool", bufs=4) as spool,
            tc.tile_pool(name="opool", bufs=3) as opool,
            tc.tile_pool(name="psab", bufs=3, space="PSUM") as psab,
            tc.tile_pool(name="pso", bufs=2, space="PSUM") as pso,
            tc.tile_pool(name="wup", bufs=1) as wup,
            tc.tile_pool(name="pwup", bufs=1, space="PSUM") as pwup,
        ):
            # PE warmup: dummy matmuls with no DMA dependency keep the PE
            # busy while the first weight/x chunks stream in, so HAM is
            # at full clock when real matmuls start.
            wu = wup.tile([P, P], BF16, tag="wu")
            nc.gpsimd.memset(wu[:], 0.0)
            pwu = pwup.tile([P, P], F32, tag="pwu")
            for _ in range(64):
                nc.tensor.matmul(pwu[:], wu[:], wu[:], start=True, stop=True)

            # one tag per weight tensor (ring of NHM slots, allocated once,
            # never cycled): fewer tags -> fewer Tile semaphores -> shorter
            # serial per-engine sem-reset chain in the kernel tail
            w1c = [wpool.tile([P, WCHC], BF16, tag="w1", bufs=NHM,
                              name=f"w1c{m}") for m in range(NHM)]
            w3c = [wpool.tile([P, WCHC], BF16, tag="w3", bufs=NHM,
                              name=f"w3c{m}") for m in range(NHM)]
            w2c = [wpool.tile([P, W2CHC], BF16, tag="w2", bufs=NHM,
                              name=f"w2c{m}") for m in range(NHM)]

            def w_slice(big, subs, m, k):
                return big[m][:, k * P:(k + 1) * P]

            def x_chunk_dma(i):
                tiles = []
                for c in range(XCHUNK):
                    xt = xpool.tile([P, KPC * TBLK], BF16, tag="x")
                    base = i * NDK * TBLK + c * KPC * TBLK
                    nc.sync.dma_start(xt[:], xp_d[:, base:base + KPC * TBLK])
                    tiles.append(xt)
                return tiles

            # DMA preamble, ordered to match PE consumption: the first
            # w1/w3 accumulation group (hm=0) needs all of x block 0 and
            # only the hm=0 chunk of w1/w3; later hm chunks arrive while
            # the previous group computes.
            # Trigger order = PE consumption order: the hm=0 w1 group
            # needs x0c0..c3 + w1c0 first; w3c0 isn't read until that
            # group finishes, so it triggers after the x chunks.
            xtiles = [None] * NTBLK
            xt0 = []
            for c in range(XCHUNK):
                xt = xpool.tile([P, KPC * TBLK], BF16, tag="x", name=f"x0c{c}")
                base = c * KPC * TBLK
                nc.sync.dma_start(xt[:], xp_d[:, base:base + KPC * TBLK])
                xt0.append(xt)
                if c == 0:
                    nc.sync.dma_start(w1c[0][:], w1_d[:, 0:WCHC])
            nc.sync.dma_start(w3c[0][:], w3_d[:, 0:WCHC])
            xtiles[0] = xt0
            for m in range(1, NHM):
                nc.sync.dma_start(w1c[m][:], w1_d[:, m * WCHC:(m + 1) * WCHC])
                nc.sync.dma_start(w3c[m][:], w3_d[:, m * WCHC:(m + 1) * WCHC])
            for m in range(NHM):
                nc.sync.dma_start(w2c[m][:], w2_d[:, m * W2CHC:(m + 1) * W2CHC])

            def x_slice(i, k):
                t = xtiles[i][k // KPC]
                return t[:, (k % KPC) * TBLK:(k % KPC + 1) * TBLK]

            for i in range(NTBLK):
                if i > 0:
                    xtiles[i] = x_chunk_dma(i)

                hts = []
                for hm in range(NHM):
                    pa = psab.tile([P, TBLK], F32, tag="pa", bufs=3)
                    pb = psab.tile([P, TBLK], F32, tag="pb", bufs=2)
                    for k in range(NDK):
                        nc.tensor.matmul(
                            pa[:], w_slice(w1c, None, hm, k), x_slice(i, k),
                            start=(k == 0), stop=(k == NDK - 1),
                        )
                    for k in range(NDK):
                        nc.tensor.matmul(
                            pb[:], w_slice(w3c, None, hm, k), x_slice(i, k),
                            start=(k == 0), stop=(k == NDK - 1),
                        )
                    sil = spool.tile([P, TBLK], BF16, tag="sil")
                    nc.scalar.activation(
                        sil[:], pa[:], mybir.ActivationFunctionType.Silu
                    )
                    ht = hpool.tile([P, TBLK], BF16, tag="h", bufs=2 * NHM)
                    nc.vector.tensor_mul(ht[:], sil[:], pb[:])
                    hts.append(ht)

                last_blk = i == NTBLK - 1
                for tm in range(NTSUB):
                    trow = i * TBLK + tm * P
                    ot = opool.tile([P, D], BF16, tag="o")
                    for dn in range(NDN):
                        dsl = dn * DBLK
                        # the very last group is split in two so the
                        # final cast + store chain after the last matmul
                        # is half as long
                        final = last_blk and tm == NTSUB - 1 and dn == NDN - 1
                        nsplit = 2 if final else 1
                        w = DBLK // nsplit
                        for s in range(nsplit):
                            ssl = dsl + s * w
                            po = pso.tile([P, w], F32, tag="po")
                            for hk in range(NHM):
                                nc.tensor.matmul(
                                    po[:],
                                    hts[hk][:, tm * P:(tm + 1) * P],
                                    w2c[hk][:, ssl:ssl + w],
                                    start=(hk == 0), stop=(hk == NHM - 1),
                                )
                            nc.vector.tensor_copy(ot[:, ssl:ssl + w], po[:])
                            if last_blk:
                                nc.sync.dma_start(
                                    out_d[trow:trow + P, ssl:ssl + w],
                                    ot[:, ssl:ssl + w],
                                )
                    if not last_blk:
                        nc.sync.dma_start(out_d[trow:trow + P, :], ot[:])

    nc.compile()
    _CACHE["nc"] = nc
    return nc


def _stage_inputs(x, w1, w2, w3):
    """Per-expert bf16 packed staging (see module docstring for layouts)."""
    bf = ml_dtypes.bfloat16
    in_maps = []
    for e in range(E):
        xT = np.ascontiguousarray(x[e].astype(bf).T)      # [D, T]
        xv = xT.reshape(NDK, P, NTBLK, TBLK)
        xp = np.ascontiguousarray(
            xv.transpose(1, 2, 0, 3)).reshape(P, XCOLS)    # [p, i, k, t]
        # w1/w3 hm-major: [p, (m, k, h128)]
        w1p = np.ascontiguousarray(
            w1[e].astype(bf).reshape(NDK, P, NHM, P).transpose(1, 2, 0, 3)
        ).reshape(P, WCOLS)
        w3p = np.ascontiguousarray(
            w3[e].astype(bf).reshape(NDK, P, NHM, P).transpose(1, 2, 0, 3)
        ).reshape(P, WCOLS)
        # w2 hm-major: [p, (m, d)]
        w2p = np.ascontiguousarray(
            w2[e].astype(bf).reshape(NHM, P, D).transpose(1, 0, 2)
        ).reshape(P, WCOLS)
        in_maps.append({"xp": xp, "w1": w1p, "w3": w3p, "w2": w2p})
    return in_maps


def kernel(x, w1, w2, w3):
    assert x.shape == (E, T, D) and w1.shape == (E, D, H)
    assert w2.shape == (E, H, D) and w3.shape == (E, D, H)
    nc = _build_module()
    in_maps = _stage_inputs(x, w1, w2, w3)
    res = run_bass_kernel_spmd(nc, in_maps, core_ids=list(range(NCORES)))
    out = np.stack([res.results[e]["out"] for e in range(E)], axis=0)
    return out.astype(np.float32)
